# revision 3
# baseline (speedup 1.0000x reference)
"""HNLoRALinear Trainium2 kernel.

out[b,s,o] = x[b] @ W^T + bias + SCALE * (x[b] @ A[b]) @ B[b]

Sharding: 8 cores = 4 batches x 2 sequence-halves. Each core computes
its [1024, 4096] output block: x-half^T is fully SBUF-resident
(128KB/partition), W^T is streamed once in 256-wide column strips.
All inputs are host-pre-transposed so the contraction dim (D_IN) lands
on SBUF partitions with no on-device transposes.

The per-sample LoRA correction + bias ride along as one extra K=17
matmul per output tile: [low ; ones] @ [SCALE*B ; bias] accumulated
into the same PSUM group as the 32 main-matmul k-chunks.

Matmuls run in float32r (TF32-like, full PE rate at moving-dim>=256,
~1.5e-4 rel err).
"""
import numpy as np

import concourse.bass as bass  # noqa: F401  (bass must import before tile)
import concourse.mybir as mybir
import concourse.tile as tile
from concourse import bacc
from concourse.bass_utils import run_bass_kernel_spmd

# Problem shapes (hardcoded per contract).
B, S, D_IN, D_OUT, R = 4, 2048, 4096, 4096, 16
SCALE = 32.0 / 16.0
SH = S // 2            # tokens per core
P = 128
KC = D_IN // P         # 32 contraction chunks
NS = 256               # out-feature strip width (>=256 keeps f32r full-rate)
N_STRIPS = D_OUT // NS # 16
M_CHUNKS = SH // P     # 8
LOW_N = 512            # token group width for the x@A matmuls
TGROUPS = SH // LOW_N  # 2
RA = R + 1             # augmented rank (lora + bias row)

_cached_nc = None


def _build():
    f32r = mybir.dt.float32r
    f32 = mybir.dt.float32
    nc = bacc.Bacc(
        "TRN2", target_bir_lowering=False, debug=False, enable_asserts=False
    )
    xt = nc.dram_tensor("xt", [D_IN, SH], f32r, kind="ExternalInput")
    wt = nc.dram_tensor("wt", [D_IN, D_OUT], f32r, kind="ExternalInput")
    apk = nc.dram_tensor("apack", [P, KC * R], f32r, kind="ExternalInput")
    bga = nc.dram_tensor("baug", [RA, D_OUT], f32r, kind="ExternalInput")
    o = nc.dram_tensor("o", [SH, D_OUT], f32, kind="ExternalOutput")

    with tile.TileContext(nc) as tc:
        with (
            tc.tile_pool(name="xp", bufs=KC) as xp,
            tc.tile_pool(name="wp", bufs=48) as wp,
            tc.tile_pool(name="cp", bufs=1) as cp,
            tc.tile_pool(name="op", bufs=4) as op,
            tc.tile_pool(name="pp", bufs=4, space="PSUM") as pp,
            tc.tile_pool(name="lp", bufs=2, space="PSUM") as lp,
        ):
            at = cp.tile([P, KC * R], f32r, name="at")
            nc.sync.dma_start(out=at[:], in_=apk.ap())
            bt = cp.tile([RA, D_OUT], f32r, name="bt")
            nc.sync.dma_start(out=bt[:], in_=bga.ap())

            xts = []
            for k in range(KC):
                xk = xp.tile([P, SH], f32r, name="xk")
                nc.sync.dma_start(out=xk[:], in_=xt.ap()[k * P : (k + 1) * P, :])
                xts.append(xk)

            # Augmented low-rank activations: rows 0..15 = x @ A, row 16 = 1.
            # (memset must start at a 32-aligned partition, so fill all 17
            # rows with ones; the copies below overwrite rows 0..15.)
            low = cp.tile([RA, SH], f32r, name="low")
            nc.gpsimd.memset(low[:].bitcast(f32), 1.0)
            for t in range(TGROUPS):
                pl = lp.tile([R, LOW_N], f32, name="pl")
                for k in range(KC):
                    nc.tensor.matmul(
                        pl[:],
                        at[:, k * R : (k + 1) * R],
                        xts[k][:, t * LOW_N : (t + 1) * LOW_N],
                        start=(k == 0),
                        stop=(k == KC - 1),
                    )
                nc.vector.tensor_copy(low[0:R, t * LOW_N : (t + 1) * LOW_N], pl[:])

            for n in range(N_STRIPS):
                wts = []
                for k in range(KC):
                    wk = wp.tile([P, NS], f32r, name="wk")
                    nc.sync.dma_start(
                        out=wk[:],
                        in_=wt.ap()[k * P : (k + 1) * P, n * NS : (n + 1) * NS],
                    )
                    wts.append(wk)
                for m in range(M_CHUNKS):
                    ps = pp.tile([P, NS], f32, name="ps")
                    for k in range(KC):
                        nc.tensor.matmul(
                            ps[:],
                            xts[k][:, m * P : (m + 1) * P],
                            wts[k][:],
                            start=(k == 0),
                            stop=False,
                        )
                    nc.tensor.matmul(
                        ps[:],
                        low[:, m * P : (m + 1) * P],
                        bt[:, n * NS : (n + 1) * NS],
                        start=False,
                        stop=True,
                    )
                    ot = op.tile([P, NS], f32, name="ot")
                    nc.vector.tensor_copy(ot[:], ps[:])
                    nc.sync.dma_start(
                        out=o.ap()[m * P : (m + 1) * P, n * NS : (n + 1) * NS],
                        in_=ot[:],
                    )
    nc.compile()
    return nc


def _get_nc():
    global _cached_nc
    if _cached_nc is None:
        _cached_nc = _build()
    return _cached_nc


def _in_maps(x, weight, bias, lora_A, lora_B):
    wt = np.ascontiguousarray(weight.T).astype(np.float32, copy=False)
    bias = bias.astype(np.float32, copy=False)
    maps = []
    for c in range(8):
        b, h = divmod(c, 2)
        xt = np.ascontiguousarray(x[b, h * SH : (h + 1) * SH, :].T).astype(
            np.float32, copy=False
        )
        apk = np.ascontiguousarray(
            lora_A[b].reshape(KC, P, R).transpose(1, 0, 2).reshape(P, KC * R)
        ).astype(np.float32, copy=False)
        baug = np.concatenate(
            [lora_B[b].astype(np.float32) * np.float32(SCALE), bias[None, :]], axis=0
        )
        maps.append({"xt": xt, "wt": wt, "apack": apk, "baug": baug})
    return maps


def kernel(x, weight, bias, lora_A, lora_B, _trace=False, _tmpdir=None):
    x = np.asarray(x, dtype=np.float32)
    weight = np.asarray(weight, dtype=np.float32)
    bias = np.asarray(bias, dtype=np.float32)
    lora_A = np.asarray(lora_A, dtype=np.float32)
    lora_B = np.asarray(lora_B, dtype=np.float32)

    nc = _get_nc()
    maps = _in_maps(x, weight, bias, lora_A, lora_B)
    res = run_bass_kernel_spmd(
        nc, maps, list(range(8)), trace=_trace, tmpdir=_tmpdir
    )
    out = np.empty((B, S, D_OUT), np.float32)
    for c in range(8):
        b, h = divmod(c, 2)
        out[b, h * SH : (h + 1) * SH, :] = res.results[c]["o"]
    if _trace:
        return out, res
    return out
